# revision 18
# baseline (speedup 1.0000x reference)
"""Trainium2 Bass kernel for the 2-module Mamba-style SSM block.

Sharding: 8 cores = 4 batches x 2 modules (core c -> batch c//2, module c%2).
Each core computes one full branch for one batch; aggregate+out_proj folded
into M_k per module; pair-wise AllReduce; host picks one core per batch.

v3: channels on partitions, fp16 cube, L chunked at LC=512.
- Depthwise causal conv is FOLDED INTO the in_proj matmul: the host
  precomputes W_j[d,e] = conv_w[k,e,j] * in_proj_w[e,d] for the 4 taps and
  the device accumulates 4 time-shifted matmuls into one PSUM tile (hs is
  loaded with a 3-column left halo).  DVE conv ops and GpSimd halo moves
  are gone; ACT applies conv bias + SiLU straight off PSUM.
- Program order per chunk: in_proj-x -> in_proj-z -> x_proj/dt -> cube, so
  ACT runs [Silu...][Exp/Ln...] with 2 activation-table loads per chunk
  instead of ~20.
- The 16-state scan runs as ONE tensor_tensor_scan per channel tile
  (state boundaries reset by zeroed a[:, n, 0]; chunk carries folded into
  u[:, :, 0]).  Decay tiles ah0/ah1 are double-buffered so ACT exps for
  tile et+1 overlap the scan of tile et.
- Carry prep (a0 = exp(A*delta0), ctmp = a0*carry) is batched per chunk.
- GpSimd does only tiny carry folds + collectives: big DVE/GpSimd streaming
  ops fully serialize on the shared SBUF port (measured), so all cube work
  stays on DVE.
- Pair AllReduce split: rows 0:1024 after chunk 1, 1024:1536 after chunk 2
  overlap compute; the last 512 rows go in two column-halves as soon as
  each out-proj column block finishes.
"""
from contextlib import ExitStack

import numpy as np

import concourse.bass as bass
import concourse.tile as tile
from concourse import bacc, mybir
from concourse.bass_utils import run_bass_kernel_spmd

FP = mybir.dt.float32
F16 = mybir.dt.float16
AX = mybir.AxisListType
OP = mybir.AluOpType
AF = mybir.ActivationFunctionType

B, L, D = 4, 2048, 1024
E, N, CW, K, R = 2048, 16, 4, 2, 64
ET = E // 128           # 16 channel tiles
DT = D // 128           # 8 d_model tiles
LC = 512                # chunk length along L
NLC = L // LC           # 4 chunks
MMF = 512               # matmul moving free size
NCORES = 8

_CACHE = {}


def _build_program():
    nc = bacc.Bacc("TRN2", target_bir_lowering=False, debug=False,
                   num_devices=NCORES)

    def din(name, shape, dt=F16):
        return nc.dram_tensor(name, list(shape), dt, kind="ExternalInput").ap()

    hsT = din("hsT", (D, L))              # hidden_states[b].T, f16
    winx = din("winx", (CW, D, E))        # tap-j x-half weights (conv folded)
    winzT = din("winzT", (D, E))          # in_proj_w z-half .T, f16
    xpT = din("xpT", (E, R + 2 * N))      # x_proj_w.T, f16
    dtwT = din("dtwT", (R, E))            # dt_w[k].T, f16
    dtb = din("dtb", (E, 1), FP)
    convb = din("convb", (E, 1), FP)
    Amat = din("Amat", (E, N), FP)        # -exp(A_log[k])
    Dp = din("Dp", (E, 1), FP)
    Mk = din("Mk", (E, D))                # (out_w @ agg_w[:, k*E:(k+1)*E]).T, f16
    out = nc.dram_tensor("out", [L, D], FP, kind="ExternalOutput").ap()

    zspill = nc.dram_tensor("zspill", [NLC, E, LC], F16).ap()
    bcspill = nc.dram_tensor("bcspill", [NLC, 2 * N, LC], F16).ap()

    with tile.TileContext(nc) as tc, ExitStack() as ctx:
        const = ctx.enter_context(tc.tile_pool(name="const", bufs=1))
        dram = ctx.enter_context(tc.tile_pool(name="dram", bufs=1, space="DRAM"))
        wpool = ctx.enter_context(tc.tile_pool(name="wpool", bufs=2))
        ch_pool = ctx.enter_context(tc.tile_pool(name="chp", bufs=1))
        xtp_pool = ctx.enter_context(tc.tile_pool(name="xtpp", bufs=2))
        hs_pool = ctx.enter_context(tc.tile_pool(name="hsp", bufs=1))
        u_pool = ctx.enter_context(tc.tile_pool(name="up", bufs=1))
        dl_pool = ctx.enter_context(tc.tile_pool(name="dlp", bufs=2))
        a0_pool = ctx.enter_context(tc.tile_pool(name="a0p", bufs=1))
        t_pool = ctx.enter_context(tc.tile_pool(name="tp", bufs=1))
        zs_pool = ctx.enter_context(tc.tile_pool(name="zsp", bufs=2))
        y_pool = ctx.enter_context(tc.tile_pool(name="yp", bufs=1))
        ev_pool = ctx.enter_context(tc.tile_pool(name="ev", bufs=1))
        zt_pool = ctx.enter_context(tc.tile_pool(name="ztp", bufs=1))
        xd_pool = ctx.enter_context(tc.tile_pool(name="xd", bufs=1))
        mk_pool = ctx.enter_context(tc.tile_pool(name="mkp", bufs=1))
        pin = ctx.enter_context(tc.tile_pool(name="pin", bufs=3, space="PSUM"))
        pxp = ctx.enter_context(tc.tile_pool(name="pxp", bufs=2, space="PSUM"))
        pdt = ctx.enter_context(tc.tile_pool(name="pdt", bufs=1, space="PSUM"))
        pout = ctx.enter_context(tc.tile_pool(name="pout", bufs=2, space="PSUM"))

        opart = dram.tile([L, D], FP)
        oshared = dram.tile([L, D], FP)

        # ---- resident constants ----
        xpT_sb = const.tile([128, ET, R + 2 * N], F16)
        nc.sync.dma_start(out=xpT_sb,
                          in_=xpT.rearrange("(a p) c -> p a c", p=128))
        dtwT_sb = const.tile([R, ET, 128], F16)
        nc.sync.dma_start(out=dtwT_sb,
                          in_=dtwT.rearrange("p (a c) -> p a c", c=128))
        Amat_sb = const.tile([128, ET, N], FP)
        nc.sync.dma_start(out=Amat_sb,
                          in_=Amat.rearrange("(a p) n -> p a n", p=128))
        dtb_sb = const.tile([128, ET, 1], FP)
        nc.sync.dma_start(out=dtb_sb, in_=dtb.rearrange("(a p) o -> p a o", p=128))
        Dp_sb = const.tile([128, ET, 1], FP)
        nc.sync.dma_start(out=Dp_sb, in_=Dp.rearrange("(a p) o -> p a o", p=128))
        convb_sb = const.tile([128, ET, 1], FP)
        nc.sync.dma_start(out=convb_sb,
                          in_=convb.rearrange("(a p) o -> p a o", p=128))
        carry = const.tile([128, ET, N], FP)
        ah0 = const.tile([128, N, LC], F16)
        ah1 = const.tile([128, N, LC], F16)
        nc.vector.memset(ah0[:, :, 0:1], 0.0)
        nc.vector.memset(ah1[:, :, 0:1], 0.0)

        def emit_ah(dlt, et):
            ah = ah0 if et % 2 == 0 else ah1
            for n in range(N):
                nc.scalar.activation(out=ah[:, n, 1:LC],
                                     in_=dlt[:, et, 1:LC], func=AF.Exp,
                                     scale=Amat_sb[:, et, n:n + 1])
            return ah

        def phase_a(lc):
            """in_proj x (conv folded) + z + x_proj + dt/softplus for chunk lc.
            Returns (xtp, dlt, Bbc, Cbc)."""
            lsl = slice(lc * LC, (lc + 1) * LC)
            hs_sb = hs_pool.tile([128, DT, CW - 1 + LC], F16, tag="hs")
            if lc == 0:
                nc.vector.memset(hs_sb[:, :, 0:CW - 1], 0.0)
                for dt_ in range(DT):
                    nc.sync.dma_start(out=hs_sb[:, dt_, CW - 1:],
                                      in_=hsT[dt_ * 128:(dt_ + 1) * 128, lsl])
            else:
                for dt_ in range(DT):
                    nc.sync.dma_start(
                        out=hs_sb[:, dt_, :],
                        in_=hsT[dt_ * 128:(dt_ + 1) * 128,
                                lc * LC - (CW - 1):(lc + 1) * LC])

            # in_proj x-half with conv folded (PE): 4 shifted matmuls
            xtp = xtp_pool.tile([128, ET, LC], F16, tag="xtp")
            psx = pxp.tile([R + 2 * N, LC], FP, tag="mmxp")
            for ct in range(ET):
                winx_ct = wpool.tile([128, CW, DT, 128], F16, tag="winx")
                for j in range(CW):
                    nc.sync.dma_start(
                        out=winx_ct[:, j],
                        in_=winx[j, :, ct * 128:(ct + 1) * 128].rearrange(
                            "(a p) c -> p a c", p=128))
                ps = pin.tile([128, MMF], FP, tag="mmin")
                for dt_ in range(DT):
                    for j in range(CW):
                        nc.tensor.matmul(ps, winx_ct[:, j, dt_, :],
                                         hs_sb[:, dt_, j:j + MMF],
                                         start=(dt_ == 0 and j == 0),
                                         stop=(dt_ == DT - 1 and j == CW - 1))
                et = ct
                nc.scalar.activation(out=xtp[:, et, :], in_=ps, func=AF.Silu,
                                     bias=convb_sb[:, et, :], scale=1.0)
                nc.tensor.matmul(psx, xpT_sb[:, et, :], xtp[:, et, :],
                                 start=(et == 0), stop=(et == ET - 1))

            xdbl = xd_pool.tile([R + 2 * N, LC], F16, tag="xdbl")
            nc.scalar.activation(out=xdbl, in_=psx, func=AF.Copy)
            nc.sync.dma_start(out=bcspill[lc], in_=xdbl[R:R + 2 * N, :])
            Bbc = ch_pool.tile([128, N, LC], F16, tag="Bbc")
            Cbc = ch_pool.tile([128, N, LC], F16, tag="Cbc")
            nc.sync.dma_start(out=Bbc, in_=bass.AP(
                tensor=bcspill.tensor, offset=lc * 2 * N * LC,
                ap=[[0, 128], [LC, N], [1, LC]]))
            nc.sync.dma_start(out=Cbc, in_=bass.AP(
                tensor=bcspill.tensor, offset=lc * 2 * N * LC + N * LC,
                ap=[[0, 128], [LC, N], [1, LC]]))

            # dt proj + softplus (before z so dlt is ready early)
            dlt = dl_pool.tile([128, ET, LC], F16, tag="dlt")
            for et in range(ET):
                psd = pdt.tile([128, LC], FP, tag="mmdt")
                nc.tensor.matmul(psd, dtwT_sb[:, et, :], xdbl[0:R, :],
                                 start=True, stop=True)
                nc.scalar.activation(out=dlt[:, et, :], in_=psd, func=AF.Exp,
                                     bias=dtb_sb[:, et, :], scale=1.0)
            for et in range(ET):
                nc.scalar.activation(out=dlt[:, et, :], in_=dlt[:, et, :],
                                     func=AF.Ln, bias=1.0)

            # in_proj z-half (PE) + silu spill
            for ct in range(ET):
                winz_ct = wpool.tile([128, DT, 128], F16, tag="winz")
                nc.sync.dma_start(
                    out=winz_ct,
                    in_=winzT[:, ct * 128:(ct + 1) * 128].rearrange(
                        "(a p) c -> p a c", p=128))
                psz = pin.tile([128, MMF], FP, tag="mmin")
                for dt_ in range(DT):
                    nc.tensor.matmul(psz, winz_ct[:, dt_, :],
                                     hs_sb[:, dt_, CW - 1:CW - 1 + MMF],
                                     start=(dt_ == 0), stop=(dt_ == DT - 1))
                zt = zt_pool.tile([128, MMF], F16, tag="zt")
                nc.scalar.activation(out=zt, in_=psz, func=AF.Silu)
                nc.sync.dma_start(
                    out=zspill[lc, ct * 128:(ct + 1) * 128, :], in_=zt)
            return xtp, dlt, Bbc, Cbc

        st = phase_a(0)
        for lc in range(NLC):
            lsl = slice(lc * LC, (lc + 1) * LC)
            xtp, dlt, Bbc, Cbc = st

            # ---- batched carry prep: ctmp_all[et] = exp(A*delta0)*carry ----
            ctmp_all = None
            if lc > 0:
                d0f = a0_pool.tile([128, ET, 1], FP, tag="d0f")
                nc.scalar.activation(out=d0f, in_=dlt[:, :, 0:1], func=AF.Copy)
                a0_all = a0_pool.tile([128, ET, N], F16, tag="a0")
                for et in range(ET):
                    nc.scalar.activation(out=a0_all[:, et, :],
                                         in_=Amat_sb[:, et, :],
                                         func=AF.Exp, scale=d0f[:, et, 0:1])
                ctmp_all = a0_pool.tile([128, ET, N], F16, tag="ctmp")
                nc.vector.tensor_tensor(out=ctmp_all, in0=a0_all, in1=carry,
                                        op=OP.mult)

            # ---- cube per channel tile ----
            ah_cur = emit_ah(dlt, 0)
            zs = zs_pool.tile([128, LC], F16, tag="zs", name="zs0")
            nc.sync.dma_start(out=zs, in_=zspill[lc, 0:128, :])
            for et in range(ET):
                delta = dlt[:, et, :]
                v = t_pool.tile([128, LC], F16, tag="v")
                nc.vector.tensor_tensor(out=v, in0=delta,
                                        in1=xtp[:, et, :], op=OP.mult)
                vb = v[:, :].rearrange("p (o t) -> p o t", o=1)
                u = u_pool.tile([128, N, LC], F16, tag="u")
                nc.vector.tensor_tensor(out=u,
                                        in0=vb.broadcast_to([128, N, LC]),
                                        in1=Bbc, op=OP.mult)
                if lc > 0:
                    # fold chunk carry into u[:, :, 0]: u0' = u0 + a0 * carry
                    ctmp3 = ctmp_all[:, et, :].rearrange("p (n o) -> p n o",
                                                         o=1)
                    nc.gpsimd.tensor_tensor(out=u[:, :, 0:1], in0=u[:, :, 0:1],
                                            in1=ctmp3, op=OP.add)
                ah = ah_cur
                if et + 1 < ET:
                    ah_cur = emit_ah(dlt, et + 1)
                nc.vector.tensor_tensor_scan(
                    out=u[:, :, :].rearrange("p n t -> p (n t)"),
                    data0=ah[:, :, :].rearrange("p n t -> p (n t)"),
                    data1=u[:, :, :].rearrange("p n t -> p (n t)"),
                    initial=0.0, op0=OP.mult, op1=OP.add)
                if lc < NLC - 1:
                    nc.scalar.activation(out=carry[:, et, :],
                                         in_=u[:, :, LC - 1], func=AF.Copy)
                # C-mult + tree reduce over n (in place on u)
                nc.vector.tensor_tensor(out=u[:, :, :], in0=u[:, :, :],
                                        in1=Cbc, op=OP.mult)
                nc.vector.tensor_tensor(out=u[:, 0:8, :], in0=u[:, 0:8, :],
                                        in1=u[:, 8:16, :], op=OP.add)
                nc.vector.tensor_tensor(out=u[:, 0:4, :], in0=u[:, 0:4, :],
                                        in1=u[:, 4:8, :], op=OP.add)
                nc.vector.tensor_tensor(out=u[:, 0:2, :], in0=u[:, 0:2, :],
                                        in1=u[:, 2:4, :], op=OP.add)
                y = u[:, 0, :]
                nc.vector.tensor_tensor(out=y, in0=u[:, 0, :], in1=u[:, 1, :],
                                        op=OP.add)
                zs_c = zs
                if et + 1 < ET:
                    zs = zs_pool.tile([128, LC], F16, tag="zs",
                                      name=f"zs{et + 1}")
                    nc.sync.dma_start(
                        out=zs,
                        in_=zspill[lc, (et + 1) * 128:(et + 2) * 128, :])
                t2 = t_pool.tile([128, LC], F16, tag="t2")
                nc.vector.scalar_tensor_tensor(out=t2, in0=xtp[:, et, :],
                                               scalar=Dp_sb[:, et, :], in1=y,
                                               op0=OP.mult, op1=OP.add)
                nc.vector.tensor_tensor(out=xtp[:, et, :],
                                        in0=t2, in1=zs_c, op=OP.mult)

            # ---- next chunk's in_proj/dt emitted BEFORE out_proj so the
            # in-order PE queue overlaps them with this chunk's cube ----
            if lc + 1 < NLC:
                st = phase_a(lc + 1)

            # ---- out_proj (PE): yf^T @ Mk, accumulated over et ----
            for dh in range(D // MMF):
                mk_sb = mk_pool.tile([128, ET, MMF], F16, tag="mk")
                nc.sync.dma_start(
                    out=mk_sb,
                    in_=Mk[:, dh * MMF:(dh + 1) * MMF].rearrange(
                        "(a p) c -> p a c", p=128))
                for tau in range(LC // 128):
                    po = pout.tile([128, MMF], FP, tag="mmo")
                    for et in range(ET):
                        nc.tensor.matmul(
                            po, xtp[:, et, tau * 128:(tau + 1) * 128],
                            mk_sb[:, et, :],
                            start=(et == 0), stop=(et == ET - 1))
                    osb = ev_pool.tile([128, MMF], FP, tag="osb")
                    nc.scalar.activation(out=osb, in_=po, func=AF.Copy)
                    nc.sync.dma_start(
                        out=opart[lc * LC + tau * 128:lc * LC + (tau + 1) * 128,
                                  dh * MMF:(dh + 1) * MMF],
                        in_=osb)
                    if lc == NLC - 1 and dh == 1 and tau % 2 == 1:
                        # tail: ship each completed 256-row block immediately
                        rsl = slice(3 * LC + (tau - 1) * 128,
                                    3 * LC + (tau + 1) * 128)
                        nc.gpsimd.collective_compute(
                            "AllReduce", OP.add,
                            replica_groups=[[0, 1], [2, 3], [4, 5], [6, 7]],
                            ins=[opart[rsl, :].opt()],
                            outs=[oshared[rsl, :].opt()])
                        nc.sync.dma_start(out=out[rsl, :],
                                          in_=oshared[rsl, :])
            if lc == 1:
                nc.gpsimd.collective_compute(
                    "AllReduce", OP.add,
                    replica_groups=[[0, 1], [2, 3], [4, 5], [6, 7]],
                    ins=[opart[0:2 * LC, :].opt()],
                    outs=[oshared[0:2 * LC, :].opt()])
                nc.sync.dma_start(out=out[0:2 * LC, :], in_=oshared[0:2 * LC, :])
            if lc == 2:
                nc.gpsimd.collective_compute(
                    "AllReduce", OP.add,
                    replica_groups=[[0, 1], [2, 3], [4, 5], [6, 7]],
                    ins=[opart[2 * LC:3 * LC, :].opt()],
                    outs=[oshared[2 * LC:3 * LC, :].opt()])
                nc.sync.dma_start(out=out[2 * LC:3 * LC, :],
                                  in_=oshared[2 * LC:3 * LC, :])

    nc.compile()
    return nc


def _get_program():
    if "nc" not in _CACHE:
        _CACHE["nc"] = _build_program()
    return _CACHE["nc"]


def kernel(**inputs):
    nc = _get_program()
    f32 = lambda a: np.ascontiguousarray(np.asarray(a), dtype=np.float32)
    f16 = lambda a: np.ascontiguousarray(np.asarray(a, dtype=np.float32),
                                         dtype=np.float16)
    hs = np.asarray(inputs["hidden_states"], dtype=np.float32)   # (B, L, D)
    in_proj_w = np.asarray(inputs["in_proj_w"], dtype=np.float32)
    xpT = f16(np.asarray(inputs["x_proj_w"], dtype=np.float32).T)
    agg_w = f32(inputs["agg_w"])
    out_w = f32(inputs["out_w"])
    conv_w = f32(inputs["conv_w"])
    conv_b = f32(inputs["conv_b"])
    dt_w = f32(inputs["dt_w"])
    dt_b = f32(inputs["dt_b"])
    A_log = f32(inputs["A_log"])
    D_param = f32(inputs["D_param"])

    winzT = f16(in_proj_w[E:2 * E].T)                      # (D, E)
    # conv folded into x-half weights: winx[j, d, e] = conv_w[k,e,j]*W[e,d]
    winxs = []
    for k in range(K):
        wx = in_proj_w[0:E][None, :, :] * conv_w[k].T[:, :, None]  # (CW, E, D)
        winxs.append(f16(np.transpose(wx, (0, 2, 1))))             # (CW, D, E)
    Mks = [f16((out_w @ agg_w[:, k * E:(k + 1) * E]).T) for k in range(K)]
    dtwTs = [f16(dt_w[k].T) for k in range(K)]
    Amats = [f32(-np.exp(A_log[k])) for k in range(K)]

    in_maps = []
    for c in range(NCORES):
        b, k = c // 2, c % 2
        in_maps.append({
            "hsT": f16(hs[b].T),
            "winx": winxs[k],
            "winzT": winzT,
            "xpT": xpT,
            "dtwT": dtwTs[k],
            "dtb": f32(dt_b[k][:, None]),
            "convb": f32(conv_b[k][:, None]),
            "Amat": Amats[k],
            "Dp": f32(D_param[k][:, None]),
            "Mk": Mks[k],
        })
    _CACHE["in_maps"] = in_maps
    res = run_bass_kernel_spmd(nc, in_maps, list(range(NCORES)))
    _CACHE["last_results"] = res.results
    out = np.empty((B, L, D), np.float32)
    for b in range(B):
        out[b] = res.results[2 * b]["out"]
    return out


# revision 21
# speedup vs baseline: 1.0421x; 1.0421x over previous
"""Trainium2 Bass kernel for the 2-module Mamba-style SSM block.

Sharding: 8 cores = 4 batches x 2 modules (core c -> batch c//2, module c%2).
Each core computes one full branch for one batch; aggregate+out_proj folded
into M_k per module; pair-wise AllReduce; host picks one core per batch.

v3: channels on partitions, fp16 cube, L chunked at LC=512.
- Depthwise causal conv is FOLDED INTO the in_proj matmul: the host
  precomputes W_j[d,e] = conv_w[k,e,j] * in_proj_w[e,d] for the 4 taps and
  the device accumulates 4 time-shifted matmuls into one PSUM tile (hs is
  loaded with a 3-column left halo).  DVE conv ops and GpSimd halo moves
  are gone; ACT applies conv bias + SiLU straight off PSUM.
- Program order per chunk: in_proj-x -> in_proj-z -> x_proj/dt -> cube, so
  ACT runs [Silu...][Exp/Ln...] with 2 activation-table loads per chunk
  instead of ~20.
- The 16-state scan runs as ONE tensor_tensor_scan per channel tile
  (state boundaries reset by zeroed a[:, n, 0]; chunk carries folded into
  u[:, :, 0]).  Decay tiles ah0/ah1 are double-buffered so ACT exps for
  tile et+1 overlap the scan of tile et.
- Carry prep (a0 = exp(A*delta0), ctmp = a0*carry) is batched per chunk.
- GpSimd does only tiny carry folds + collectives: big DVE/GpSimd streaming
  ops fully serialize on the shared SBUF port (measured), so all cube work
  stays on DVE.
- Pair AllReduce split: rows 0:1024 after chunk 1, 1024:1536 after chunk 2
  overlap compute; the last 512 rows go in two column-halves as soon as
  each out-proj column block finishes.
"""
from contextlib import ExitStack

import numpy as np

import concourse.bass as bass
import concourse.tile as tile
from concourse import bacc, mybir
from concourse.bass_utils import run_bass_kernel_spmd

FP = mybir.dt.float32
F16 = mybir.dt.float16
AX = mybir.AxisListType
OP = mybir.AluOpType
AF = mybir.ActivationFunctionType

B, L, D = 4, 2048, 1024
E, N, CW, K, R = 2048, 16, 4, 2, 64
ET = E // 128           # 16 channel tiles
DT = D // 128           # 8 d_model tiles
LC = 512                # chunk length along L
NLC = L // LC           # 4 chunks
MMF = 512               # matmul moving free size
NCORES = 8

_CACHE = {}


def _steer_act_tables():
    """Steer the act-table chooser: hide exp/ln from the early single-func
    sets so the greedy first-containing-set pick lands on the combined
    natural_log_exp_and_others table. Kills the Exp<->Ln table ping-pong
    (each reload costs 1.28us on ACT). Execution is unaffected: the emitted
    set id still indexes act_info.json, and that set really contains both."""
    if _CACHE.get("steered"):
        return
    import concourse.bacc as bacc_mod
    orig = bacc_mod.get_activation_tables

    def patched(arch):
        t = dict(orig(arch))
        for name in ("exp_and_others", "natural_log"):
            if name in t:
                t[name] = t[name] - {AF.Exp, AF.Ln}
        return t

    bacc_mod.get_activation_tables = patched
    _CACHE["steered"] = True


def _build_program():
    _steer_act_tables()
    nc = bacc.Bacc("TRN2", target_bir_lowering=False, debug=False,
                   num_devices=NCORES)

    def din(name, shape, dt=F16):
        return nc.dram_tensor(name, list(shape), dt, kind="ExternalInput").ap()

    hsT = din("hsT", (D, L))              # hidden_states[b].T, f16
    winx = din("winx", (CW, D, E))        # tap-j x-half weights (conv folded)
    winzT = din("winzT", (D, E))          # in_proj_w z-half .T, f16
    xpT = din("xpT", (E, R + 2 * N))      # x_proj_w.T, f16
    dtwT = din("dtwT", (R, E))            # dt_w[k].T, f16
    dtb = din("dtb", (E, 1), FP)
    convb = din("convb", (E, 1), FP)
    Amat = din("Amat", (E, N), FP)        # -exp(A_log[k])
    Dp = din("Dp", (E, 1), FP)
    Mk = din("Mk", (E, D))                # (out_w @ agg_w[:, k*E:(k+1)*E]).T, f16
    out = nc.dram_tensor("out", [L, D], FP, kind="ExternalOutput").ap()

    zspill = nc.dram_tensor("zspill", [NLC, E, LC], F16).ap()
    bcspill = nc.dram_tensor("bcspill", [NLC, 2 * N, LC], F16).ap()

    with tile.TileContext(nc) as tc, ExitStack() as ctx:
        const = ctx.enter_context(tc.tile_pool(name="const", bufs=1))
        dram = ctx.enter_context(tc.tile_pool(name="dram", bufs=1, space="DRAM"))
        wpool = ctx.enter_context(tc.tile_pool(name="wpool", bufs=2))
        ch_pool = ctx.enter_context(tc.tile_pool(name="chp", bufs=1))
        xtp_pool = ctx.enter_context(tc.tile_pool(name="xtpp", bufs=2))
        hs_pool = ctx.enter_context(tc.tile_pool(name="hsp", bufs=1))
        u_pool = ctx.enter_context(tc.tile_pool(name="up", bufs=1))
        dl_pool = ctx.enter_context(tc.tile_pool(name="dlp", bufs=2))
        a0_pool = ctx.enter_context(tc.tile_pool(name="a0p", bufs=1))
        t_pool = ctx.enter_context(tc.tile_pool(name="tp", bufs=1))
        zs_pool = ctx.enter_context(tc.tile_pool(name="zsp", bufs=2))
        y_pool = ctx.enter_context(tc.tile_pool(name="yp", bufs=1))
        ev_pool = ctx.enter_context(tc.tile_pool(name="ev", bufs=1))
        zt_pool = ctx.enter_context(tc.tile_pool(name="ztp", bufs=1))
        xd_pool = ctx.enter_context(tc.tile_pool(name="xd", bufs=1))
        mk_pool = ctx.enter_context(tc.tile_pool(name="mkp", bufs=1))
        pin = ctx.enter_context(tc.tile_pool(name="pin", bufs=2, space="PSUM"))
        pxp = ctx.enter_context(tc.tile_pool(name="pxp", bufs=2, space="PSUM"))
        pdt = ctx.enter_context(tc.tile_pool(name="pdt", bufs=2, space="PSUM"))
        pout = ctx.enter_context(tc.tile_pool(name="pout", bufs=2, space="PSUM"))

        opart = dram.tile([L, D], FP)
        oshared = dram.tile([L, D], FP)

        # ---- resident constants ----
        xpT_sb = const.tile([128, ET, R + 2 * N], F16)
        nc.sync.dma_start(out=xpT_sb,
                          in_=xpT.rearrange("(a p) c -> p a c", p=128))
        dtwT_sb = const.tile([R, ET, 128], F16)
        nc.sync.dma_start(out=dtwT_sb,
                          in_=dtwT.rearrange("p (a c) -> p a c", c=128))
        Amat_sb = const.tile([128, ET, N], FP)
        nc.sync.dma_start(out=Amat_sb,
                          in_=Amat.rearrange("(a p) n -> p a n", p=128))
        dtb_sb = const.tile([128, ET, 1], FP)
        nc.sync.dma_start(out=dtb_sb, in_=dtb.rearrange("(a p) o -> p a o", p=128))
        Dp_sb = const.tile([128, ET, 1], FP)
        nc.sync.dma_start(out=Dp_sb, in_=Dp.rearrange("(a p) o -> p a o", p=128))
        convb_sb = const.tile([128, ET, 1], FP)
        nc.sync.dma_start(out=convb_sb,
                          in_=convb.rearrange("(a p) o -> p a o", p=128))
        carry = const.tile([128, ET, N], FP)
        ah0 = const.tile([128, N, LC], F16)
        ah1 = const.tile([128, N, LC], F16)
        nc.vector.memset(ah0[:, :, 0:1], 0.0)
        nc.vector.memset(ah1[:, :, 0:1], 0.0)

        def emit_ah(dlt, et):
            ah = ah0 if et % 2 == 0 else ah1
            for n in range(N):
                nc.scalar.activation(out=ah[:, n, 1:LC],
                                     in_=dlt[:, et, 1:LC], func=AF.Exp,
                                     scale=Amat_sb[:, et, n:n + 1])
            return ah

        def phase_a(lc):
            """in_proj x (conv folded) + z + x_proj + dt/softplus for chunk lc.
            Returns (xtp, dlt, Bbc, Cbc)."""
            lsl = slice(lc * LC, (lc + 1) * LC)
            hs_sb = hs_pool.tile([128, DT, CW - 1 + LC], F16, tag="hs")
            if lc == 0:
                nc.vector.memset(hs_sb[:, :, 0:CW - 1], 0.0)
                for dt_ in range(DT):
                    nc.sync.dma_start(out=hs_sb[:, dt_, CW - 1:],
                                      in_=hsT[dt_ * 128:(dt_ + 1) * 128, lsl])
            else:
                for dt_ in range(DT):
                    nc.sync.dma_start(
                        out=hs_sb[:, dt_, :],
                        in_=hsT[dt_ * 128:(dt_ + 1) * 128,
                                lc * LC - (CW - 1):(lc + 1) * LC])

            # in_proj x-half with conv folded (PE): 4 shifted matmuls
            xtp = xtp_pool.tile([128, ET, LC], F16, tag="xtp")
            psx = pxp.tile([R + 2 * N, LC], FP, tag="mmxp")
            for ct in range(ET):
                winx_ct = wpool.tile([128, CW, DT, 128], F16, tag="winx")
                for j in range(CW):
                    nc.sync.dma_start(
                        out=winx_ct[:, j],
                        in_=winx[j, :, ct * 128:(ct + 1) * 128].rearrange(
                            "(a p) c -> p a c", p=128))
                ps = pin.tile([128, MMF], FP, tag="mmin")
                for dt_ in range(DT):
                    for j in range(CW):
                        nc.tensor.matmul(ps, winx_ct[:, j, dt_, :],
                                         hs_sb[:, dt_, j:j + MMF],
                                         start=(dt_ == 0 and j == 0),
                                         stop=(dt_ == DT - 1 and j == CW - 1))
                et = ct
                nc.scalar.activation(out=xtp[:, et, :], in_=ps, func=AF.Silu,
                                     bias=convb_sb[:, et, :], scale=1.0)
                nc.tensor.matmul(psx, xpT_sb[:, et, :], xtp[:, et, :],
                                 start=(et == 0), stop=(et == ET - 1))

            xdbl = xd_pool.tile([R + 2 * N, LC], F16, tag="xdbl")
            nc.scalar.activation(out=xdbl, in_=psx, func=AF.Copy)
            nc.sync.dma_start(out=bcspill[lc], in_=xdbl[R:R + 2 * N, :])
            Bbc = ch_pool.tile([128, N, LC], F16, tag="Bbc")
            Cbc = ch_pool.tile([128, N, LC], F16, tag="Cbc")
            nc.sync.dma_start(out=Bbc, in_=bass.AP(
                tensor=bcspill.tensor, offset=lc * 2 * N * LC,
                ap=[[0, 128], [LC, N], [1, LC]]))
            nc.sync.dma_start(out=Cbc, in_=bass.AP(
                tensor=bcspill.tensor, offset=lc * 2 * N * LC + N * LC,
                ap=[[0, 128], [LC, N], [1, LC]]))

            # dt proj + softplus, interleaved per et (before z so dlt[0..]
            # is ready early; Exp/Ln share one act table via _steer_act_tables)
            dlt = dl_pool.tile([128, ET, LC], F16, tag="dlt")
            for et in range(ET):
                psd = pdt.tile([128, LC], FP, tag="mmdt")
                nc.tensor.matmul(psd, dtwT_sb[:, et, :], xdbl[0:R, :],
                                 start=True, stop=True)
                nc.scalar.activation(out=dlt[:, et, :], in_=psd, func=AF.Exp,
                                     bias=dtb_sb[:, et, :], scale=1.0)
                nc.scalar.activation(out=dlt[:, et, :], in_=dlt[:, et, :],
                                     func=AF.Ln, bias=1.0)

            # in_proj z-half (PE) + silu spill
            for ct in range(ET):
                winz_ct = wpool.tile([128, DT, 128], F16, tag="winz")
                nc.sync.dma_start(
                    out=winz_ct,
                    in_=winzT[:, ct * 128:(ct + 1) * 128].rearrange(
                        "(a p) c -> p a c", p=128))
                psz = pin.tile([128, MMF], FP, tag="mmin")
                for dt_ in range(DT):
                    nc.tensor.matmul(psz, winz_ct[:, dt_, :],
                                     hs_sb[:, dt_, CW - 1:CW - 1 + MMF],
                                     start=(dt_ == 0), stop=(dt_ == DT - 1))
                zt = zt_pool.tile([128, MMF], F16, tag="zt")
                nc.scalar.activation(out=zt, in_=psz, func=AF.Silu)
                nc.sync.dma_start(
                    out=zspill[lc, ct * 128:(ct + 1) * 128, :], in_=zt)
            return xtp, dlt, Bbc, Cbc

        st = phase_a(0)
        for lc in range(NLC):
            lsl = slice(lc * LC, (lc + 1) * LC)
            xtp, dlt, Bbc, Cbc = st

            # ---- batched carry prep: ctmp_all[et] = exp(A*delta0)*carry ----
            ctmp_all = None
            if lc > 0:
                d0f = a0_pool.tile([128, ET, 1], FP, tag="d0f")
                nc.scalar.activation(out=d0f, in_=dlt[:, :, 0:1], func=AF.Copy)
                a0_all = a0_pool.tile([128, ET, N], F16, tag="a0")
                for et in range(ET):
                    nc.scalar.activation(out=a0_all[:, et, :],
                                         in_=Amat_sb[:, et, :],
                                         func=AF.Exp, scale=d0f[:, et, 0:1])
                ctmp_all = a0_pool.tile([128, ET, N], F16, tag="ctmp")
                nc.vector.tensor_tensor(out=ctmp_all, in0=a0_all, in1=carry,
                                        op=OP.mult)

            # ---- cube per channel tile ----
            ah_cur = emit_ah(dlt, 0)
            zs = zs_pool.tile([128, LC], F16, tag="zs", name="zs0")
            nc.sync.dma_start(out=zs, in_=zspill[lc, 0:128, :])
            for et in range(ET):
                delta = dlt[:, et, :]
                v = t_pool.tile([128, LC], F16, tag="v")
                nc.vector.tensor_tensor(out=v, in0=delta,
                                        in1=xtp[:, et, :], op=OP.mult)
                vb = v[:, :].rearrange("p (o t) -> p o t", o=1)
                u = u_pool.tile([128, N, LC], F16, tag="u")
                nc.vector.tensor_tensor(out=u,
                                        in0=vb.broadcast_to([128, N, LC]),
                                        in1=Bbc, op=OP.mult)
                if lc > 0:
                    # fold chunk carry into u[:, :, 0]: u0' = u0 + a0 * carry
                    ctmp3 = ctmp_all[:, et, :].rearrange("p (n o) -> p n o",
                                                         o=1)
                    nc.gpsimd.tensor_tensor(out=u[:, :, 0:1], in0=u[:, :, 0:1],
                                            in1=ctmp3, op=OP.add)
                ah = ah_cur
                if et + 1 < ET:
                    ah_cur = emit_ah(dlt, et + 1)
                nc.vector.tensor_tensor_scan(
                    out=u[:, :, :].rearrange("p n t -> p (n t)"),
                    data0=ah[:, :, :].rearrange("p n t -> p (n t)"),
                    data1=u[:, :, :].rearrange("p n t -> p (n t)"),
                    initial=0.0, op0=OP.mult, op1=OP.add)
                if lc < NLC - 1:
                    nc.scalar.activation(out=carry[:, et, :],
                                         in_=u[:, :, LC - 1], func=AF.Copy)
                # C-mult + tree reduce over n (in place on u)
                nc.vector.tensor_tensor(out=u[:, :, :], in0=u[:, :, :],
                                        in1=Cbc, op=OP.mult)
                nc.vector.tensor_tensor(out=u[:, 0:8, :], in0=u[:, 0:8, :],
                                        in1=u[:, 8:16, :], op=OP.add)
                nc.vector.tensor_tensor(out=u[:, 0:4, :], in0=u[:, 0:4, :],
                                        in1=u[:, 4:8, :], op=OP.add)
                nc.vector.tensor_tensor(out=u[:, 0:2, :], in0=u[:, 0:2, :],
                                        in1=u[:, 2:4, :], op=OP.add)
                y = u[:, 0, :]
                nc.vector.tensor_tensor(out=y, in0=u[:, 0, :], in1=u[:, 1, :],
                                        op=OP.add)
                zs_c = zs
                if et + 1 < ET:
                    zs = zs_pool.tile([128, LC], F16, tag="zs",
                                      name=f"zs{et + 1}")
                    nc.sync.dma_start(
                        out=zs,
                        in_=zspill[lc, (et + 1) * 128:(et + 2) * 128, :])
                t2 = t_pool.tile([128, LC], F16, tag="t2")
                nc.vector.scalar_tensor_tensor(out=t2, in0=xtp[:, et, :],
                                               scalar=Dp_sb[:, et, :], in1=y,
                                               op0=OP.mult, op1=OP.add)
                nc.vector.tensor_tensor(out=xtp[:, et, :],
                                        in0=t2, in1=zs_c, op=OP.mult)

            # ---- next chunk's in_proj/dt emitted BEFORE out_proj so the
            # in-order PE queue overlaps them with this chunk's cube ----
            if lc + 1 < NLC:
                st = phase_a(lc + 1)

            # ---- out_proj (PE): yf^T @ Mk, accumulated over et ----
            for dh in range(D // MMF):
                mk_sb = mk_pool.tile([128, ET, MMF], F16, tag="mk")
                nc.sync.dma_start(
                    out=mk_sb,
                    in_=Mk[:, dh * MMF:(dh + 1) * MMF].rearrange(
                        "(a p) c -> p a c", p=128))
                for tau in range(LC // 128):
                    po = pout.tile([128, MMF], FP, tag="mmo")
                    for et in range(ET):
                        nc.tensor.matmul(
                            po, xtp[:, et, tau * 128:(tau + 1) * 128],
                            mk_sb[:, et, :],
                            start=(et == 0), stop=(et == ET - 1))
                    osb = ev_pool.tile([128, MMF], FP, tag="osb")
                    nc.scalar.activation(out=osb, in_=po, func=AF.Copy)
                    nc.sync.dma_start(
                        out=opart[lc * LC + tau * 128:lc * LC + (tau + 1) * 128,
                                  dh * MMF:(dh + 1) * MMF],
                        in_=osb)
                    if lc == NLC - 1 and dh == 1 and tau % 2 == 1:
                        # tail: ship each completed 256-row block immediately
                        rsl = slice(3 * LC + (tau - 1) * 128,
                                    3 * LC + (tau + 1) * 128)
                        nc.gpsimd.collective_compute(
                            "AllReduce", OP.add,
                            replica_groups=[[0, 1], [2, 3], [4, 5], [6, 7]],
                            ins=[opart[rsl, :].opt()],
                            outs=[oshared[rsl, :].opt()])
                        nc.sync.dma_start(out=out[rsl, :],
                                          in_=oshared[rsl, :])
            if lc == 1:
                nc.gpsimd.collective_compute(
                    "AllReduce", OP.add,
                    replica_groups=[[0, 1], [2, 3], [4, 5], [6, 7]],
                    ins=[opart[0:2 * LC, :].opt()],
                    outs=[oshared[0:2 * LC, :].opt()])
                nc.sync.dma_start(out=out[0:2 * LC, :], in_=oshared[0:2 * LC, :])
            if lc == 2:
                nc.gpsimd.collective_compute(
                    "AllReduce", OP.add,
                    replica_groups=[[0, 1], [2, 3], [4, 5], [6, 7]],
                    ins=[opart[2 * LC:3 * LC, :].opt()],
                    outs=[oshared[2 * LC:3 * LC, :].opt()])
                nc.sync.dma_start(out=out[2 * LC:3 * LC, :],
                                  in_=oshared[2 * LC:3 * LC, :])

    nc.compile()
    return nc


def _get_program():
    if "nc" not in _CACHE:
        _CACHE["nc"] = _build_program()
    return _CACHE["nc"]


def kernel(**inputs):
    nc = _get_program()
    f32 = lambda a: np.ascontiguousarray(np.asarray(a), dtype=np.float32)
    f16 = lambda a: np.ascontiguousarray(np.asarray(a, dtype=np.float32),
                                         dtype=np.float16)
    hs = np.asarray(inputs["hidden_states"], dtype=np.float32)   # (B, L, D)
    in_proj_w = np.asarray(inputs["in_proj_w"], dtype=np.float32)
    xpT = f16(np.asarray(inputs["x_proj_w"], dtype=np.float32).T)
    agg_w = f32(inputs["agg_w"])
    out_w = f32(inputs["out_w"])
    conv_w = f32(inputs["conv_w"])
    conv_b = f32(inputs["conv_b"])
    dt_w = f32(inputs["dt_w"])
    dt_b = f32(inputs["dt_b"])
    A_log = f32(inputs["A_log"])
    D_param = f32(inputs["D_param"])

    winzT = f16(in_proj_w[E:2 * E].T)                      # (D, E)
    # conv folded into x-half weights: winx[j, d, e] = conv_w[k,e,j]*W[e,d]
    winxs = []
    for k in range(K):
        wx = in_proj_w[0:E][None, :, :] * conv_w[k].T[:, :, None]  # (CW, E, D)
        winxs.append(f16(np.transpose(wx, (0, 2, 1))))             # (CW, D, E)
    Mks = [f16((out_w @ agg_w[:, k * E:(k + 1) * E]).T) for k in range(K)]
    dtwTs = [f16(dt_w[k].T) for k in range(K)]
    Amats = [f32(-np.exp(A_log[k])) for k in range(K)]

    in_maps = []
    for c in range(NCORES):
        b, k = c // 2, c % 2
        in_maps.append({
            "hsT": f16(hs[b].T),
            "winx": winxs[k],
            "winzT": winzT,
            "xpT": xpT,
            "dtwT": dtwTs[k],
            "dtb": f32(dt_b[k][:, None]),
            "convb": f32(conv_b[k][:, None]),
            "Amat": Amats[k],
            "Dp": f32(D_param[k][:, None]),
            "Mk": Mks[k],
        })
    _CACHE["in_maps"] = in_maps
    res = run_bass_kernel_spmd(nc, in_maps, list(range(NCORES)))
    _CACHE["last_results"] = res.results
    out = np.empty((B, L, D), np.float32)
    for b in range(B):
        out[b] = res.results[2 * b]["out"]
    return out
